# revision 26
# baseline (speedup 1.0000x reference)
"""MoE (top-2 of 8 experts, d=1024) — expert-parallel Bass kernel, 8 trn2 cores.

Strategy: expert-parallel (core e hosts expert e). Host computes the gate and
top-2 dispatch (0.2% of FLOPs); each core runs its expert's MLP over the
tokens routed to it (padded to capacity C, chunk-major transposed layout);
host scatter-adds the two expert contributions per token.

Device pipeline per 512-token chunk: yT = (relu(W1^T xT + b1)^T W2 + b2) * w
with f32r matmuls (full PE rate, measured ~218ns per 128x128x512 MM in-kernel;
bf16 MMs measured slower in sustained kernels despite the cost model).

Key optimizations over the plain f32r baseline:
 - W1/W2/x live in HBM as bf16 (half the DMA bytes) and are upconverted to
   f32 by the otherwise-idle ScalarE (ACT) engine right after each DMA slice
   lands. This unstarves the first chunk: the PE's first matmul needs only
   ~1.3MB (x0 + w1 mc-block 0) instead of 6MB of f32.
 - w1 is DMA'd mc-major (column blocks spanning all kc) so h(0, mc) can start
   as soon as block mc arrives; w2/x stream behind compute.
 - y output is written as bf16 (DVE converts for free; halves writeback).
 - Software pipelining interleaves chunk n's y-phase with chunk n+1's h-phase
   at mc granularity, hiding the PSUM->relu->SBUF latency bubble.
 - Combine weights broadcast across partitions on-device (K=1 ones matmul,
   all chunks upfront — also warms the PE p-state at iteration start).
 - Optional load balancing: primary capacity C_p=2048 plus n_sp 128-token
   spill slots per core carrying overflow tokens of hot experts with their
   own (bf16, unconverted — small free dim) expert weights.
"""

import numpy as np
import ml_dtypes

import concourse.bass as bass
import concourse.mybir as mybir
import concourse.tile as tile
from concourse import bacc
from concourse.bass_utils import run_bass_kernel_spmd

# Problem shapes (hardcoded per contract)
D = 1024  # d_model == d_hidden
N_EXPERTS = 8
TOP_K = 2
N_CORES = 8
B, T = 4, 2048
N_TOKENS = B * T

F32 = mybir.dt.float32
F32R = mybir.dt.float32r
BF16 = mybir.dt.bfloat16
NPBF = ml_dtypes.bfloat16
KC = D // 128  # contraction chunks (8)
MC = D // 128  # output-feature chunks (8)
NT = 512       # max tokens per primary chunk (f32 moving-operand max)
C_P = 2048     # primary capacity per core
SIZES = [512, 512, 256, 256, 256, 256]  # primary chunk sizes (sum == C_P)
S_SP = 128     # spill slot size


def build_moe_kernel(n_sp: int, repeat: int = 1, pipe: bool = True,
                     sp_sz: int = S_SP, n_warm: int = 24,
                     sp_last: bool = False) -> bacc.Bacc:
    """MoE expert kernel: C_P/NT primary 512-token chunks (core's own expert,
    f32r MMs, bf16-in-HBM weights+x upconverted on ACT) plus n_sp 128-token
    bf16 spill chunks (foreign experts, bf16 weights used directly)."""
    SP = n_sp * sp_sz
    C = C_P + SP

    nc = bacc.Bacc("TRN2", target_bir_lowering=False, debug=False,
                   num_devices=N_CORES)

    xT = nc.dram_tensor("xT", [D, C_P], F32R, kind="ExternalInput")
    xB0 = nc.dram_tensor("xB0", [D, SIZES[0]], BF16, kind="ExternalInput")
    xB1 = nc.dram_tensor("xB1", [D, SIZES[1]], BF16, kind="ExternalInput")
    wbT = nc.dram_tensor("wbT", [128, C], BF16, kind="ExternalInput")
    w1 = nc.dram_tensor("w1", [D, D], BF16, kind="ExternalInput")
    b1 = nc.dram_tensor("b1", [D], F32, kind="ExternalInput")
    w2 = nc.dram_tensor("w2", [D, D], BF16, kind="ExternalInput")
    b2 = nc.dram_tensor("b2", [D], F32, kind="ExternalInput")
    yT = nc.dram_tensor("yT", [D, C_P], BF16, kind="ExternalOutput")
    if n_sp:
        xSp = nc.dram_tensor("xSp", [D, SP], BF16, kind="ExternalInput")
        w1s = nc.dram_tensor("w1s", [n_sp, D, D], BF16, kind="ExternalInput")
        b1s = nc.dram_tensor("b1s", [n_sp, D], F32, kind="ExternalInput")
        w2s = nc.dram_tensor("w2s", [n_sp, D, D], BF16, kind="ExternalInput")
        b2s = nc.dram_tensor("b2s", [n_sp, D], F32, kind="ExternalInput")
        ySp = nc.dram_tensor("ySp", [1, D, SP], BF16, kind="ExternalOutput")

    # DRAM views: partition-dim-first tilings
    xT_v = xT.ap().rearrange("(kc kp) t -> kp kc t", kc=KC)   # [128, KC, C_P]
    xB0_v = xB0.ap().rearrange("(kc kp) t -> kp kc t", kc=KC)
    xB1_v = xB1.ap().rearrange("(kc kp) t -> kp kc t", kc=KC)
    w1_v = w1.ap().rearrange("(kc kp) m -> kp kc m", kc=KC)   # kc-major
    w2_v = w2.ap().rearrange("(kc kp) (mc mb) -> kp kc mc mb", kc=KC, mc=MC)
    b1_v = b1.ap().rearrange("(mc mp) -> mp mc", mc=MC)          # [128, MC]
    b2_v = b2.ap().rearrange("(mc mp) -> mp mc", mc=MC)
    yT_v = yT.ap().rearrange("(mc mp) t -> mp mc t", mc=MC)   # [128, MC, C_P]
    if n_sp:
        xSp_v = xSp.ap().rearrange("(kc kp) t -> kp kc t", kc=KC)
        w1s_v = w1s.ap().rearrange("e (kc kp) m -> e kp kc m", kc=KC)
        w2s_v = w2s.ap().rearrange("e (kc kp) m -> e kp kc m", kc=KC)
        b1s_v = b1s.ap().rearrange("e (mc mp) -> e mp mc", mc=MC)
        b2s_v = b2s.ap().rearrange("e (mc mp) -> e mp mc", mc=MC)
        ySp_v = ySp.ap().rearrange("o (mc mp) t -> o mp mc t", mc=MC)

    with tile.TileContext(nc) as tc:
        with (
            tc.tile_pool(name="weights", bufs=1) as wpool,
            tc.tile_pool(name="wstage", bufs=1) as spool,
            tc.tile_pool(name="consts", bufs=1) as cpool,
            tc.tile_pool(name="xstage", bufs=4) as xspool,
            tc.tile_pool(name="xin", bufs=3) as xpool,
            tc.tile_pool(name="hmid", bufs=2) as hpool,
            tc.tile_pool(name="yout", bufs=2) as ypool,
            tc.tile_pool(name="ph", bufs=4, space="PSUM") as phpool,
            tc.tile_pool(name="py", bufs=3, space="PSUM") as pypool,
        ):
            from contextlib import nullcontext
            loop_cm = (
                tc.For_i(0, repeat, 1,
                         hint_engines=(mybir.EngineType.PE,
                                       mybir.EngineType.Activation,
                                       mybir.EngineType.DVE,
                                       mybir.EngineType.SP))
                if repeat > 1 else nullcontext()
            )
            with loop_cm:
                w1_sb = wpool.tile([128, KC, D], F32R, tag="w1")
                w2_sb = wpool.tile([128, KC, D], F32R, tag="w2")
                b1_sb = cpool.tile([128, MC], F32, tag="b1")
                b2_sb = cpool.tile([128, MC], F32, tag="b2")
                wb_full = cpool.tile([128, C], BF16, tag="wbf")
                x0 = xpool.tile([128, KC, SIZES[0]], BF16, tag="x",
                                name="x0")
                x1 = xpool.tile([128, KC, SIZES[1]], BF16, tag="x",
                                name="x1")
                if n_sp:
                    w1s_sb = [wpool.tile([128, KC, D], BF16, tag=f"w1s{j}",
                                         name=f"w1s{j}")
                              for j in range(n_sp)]
                    w2s_sb = [wpool.tile([128, KC, D], BF16, tag=f"w2s{j}",
                                         name=f"w2s{j}")
                              for j in range(n_sp)]
                    b1s_sb = [cpool.tile([128, MC], F32, tag=f"b1s{j}",
                                         name=f"b1s{j}")
                              for j in range(n_sp)]
                    b2s_sb = [cpool.tile([128, MC], F32, tag=f"b2s{j}",
                                         name=f"b2s{j}")
                              for j in range(n_sp)]
                    xs_sb = xpool.tile([128, KC, SP], BF16, tag="xs")

                nc.sync.dma_start(b1_sb[:], b1_v)
                nc.sync.dma_start(b2_sb[:], b2_v)
                if n_sp:
                    for j in range(n_sp):
                        nc.sync.dma_start(b1s_sb[j][:], b1s_v[j])
                        nc.sync.dma_start(b2s_sb[j][:], b2s_v[j])

                def conv_x(xt, kc, src, sz):
                    nc.sync.dma_start(xt[:, kc:kc + 2, :sz], src)

                # Prologue: x0 kc-slices (bf16) interleaved with mc-major
                # w1 staging blocks (bf16). Chunks 0/1 run straight off the
                # bf16 staging tiles; the f32 upconversion happens later on
                # the idle ACT engine for chunks 2+.
                w1stg = spool.tile([128, KC, D], BF16, tag="w1stg")
                w2stg = spool.tile([128, MC, KC, 128], BF16, tag="w2stg")
                # wb broadcast comes precomputed from the host: 4 column
                # splits on the Pool queue (idle engine, parallel dispatcher)
                q = C // 4
                for i in range(4):
                    nc.gpsimd.dma_start(wb_full[:, i * q:(i + 1) * q],
                                        wbT.ap()[:, i * q:(i + 1) * q])
                # x0 kc slices on SP; w1 kc-slice halves alternating between
                # ACT and Pool queues — three parallel dispatchers so the
                # cold chunk's operands land with minimal serialization.
                hd = D // 2
                qd = D // 4
                for i in range(KC):
                    if i == 0:
                        nc.sync.dma_start(x0[:, 0, 0:SIZES[0] // 2],
                                          xB0_v[:, 0, 0:SIZES[0] // 2])
                        nc.sync.dma_start(x0[:, 0, SIZES[0] // 2:],
                                          xB0_v[:, 0, SIZES[0] // 2:])
                    else:
                        nc.sync.dma_start(x0[:, i, :], xB0_v[:, i, :])
                    eng = nc.scalar if i % 2 == 0 else nc.gpsimd
                    if i < 2:
                        for p in range(4):
                            eng.dma_start(w1stg[:, i, p * qd:(p + 1) * qd],
                                          w1_v[:, i, p * qd:(p + 1) * qd])
                    else:
                        eng.dma_start(w1stg[:, i, 0:hd], w1_v[:, i, 0:hd])
                        eng.dma_start(w1stg[:, i, hd:D], w1_v[:, i, hd:D])

                def emit_x1_w2stg():
                    for i in range(KC):
                        nc.sync.dma_start(x1[:, i, :], xB1_v[:, i, :])
                        nc.gpsimd.dma_start(w2stg[:, i, :, :],
                                            w2_v[:, :, i, :])

                def emit_converts_and_spill():
                    for i in range(KC):
                        nc.scalar.activation(
                            w1_sb[:, i, :], w1stg[:, i, :],
                            mybir.ActivationFunctionType.Identity)
                    for i in range(MC):
                        nc.scalar.activation(
                            w2_sb[:, :, bass.ts(i, 128)], w2stg[:, i, :, :],
                            mybir.ActivationFunctionType.Identity)
                    if n_sp:
                        h_kc = KC // 2
                        for i in (0, h_kc):
                            nc.sync.dma_start(xs_sb[:, i:i + h_kc, :],
                                              xSp_v[:, i:i + h_kc, :])
                        qk = KC // 4
                        for j in range(n_sp):
                            for i in range(0, KC, qk):
                                nc.gpsimd.dma_start(
                                    w1s_sb[j][:, i:i + qk, :],
                                    w1s_v[j][:, i:i + qk, :])
                        for j in range(n_sp):
                            for i in range(0, KC, qk):
                                nc.gpsimd.dma_start(
                                    w2s_sb[j][:, i:i + qk, :],
                                    w2s_v[j][:, i:i + qk, :])

                # Chunk descriptors: primaries (variable sizes) with the
                # spill chunks inserted before the last small primary so the
                # iteration tail is a cheap 256-token chunk.
                def stg1_ap(kc, mc):
                    return w1stg[:, kc, bass.ts(mc, 128)]

                def stg2_ap(kc, mc):
                    return w2stg[:, mc, kc, :]

                def sb_ap(w_sb):
                    return lambda kc, mc: w_sb[:, kc, bass.ts(mc, 128)]

                prim_chunks = []
                offs = [sum(SIZES[:i]) for i in range(len(SIZES))]
                for n, (o, s) in enumerate(zip(offs, SIZES)):
                    cold = n < 2
                    prim_chunks.append(dict(
                        sz=s, off=o, xv=xT_v[:, :, o:o + s],
                        yv=yT_v[:, :, o:o + s], hdt=BF16 if cold else F32R,
                        w1ap=stg1_ap if cold else sb_ap(w1_sb),
                        w2ap=stg2_ap if cold else sb_ap(w2_sb),
                        b1t=b1_sb, b2t=b2_sb, prim=n))
                sp_chunks = []
                for j in range(n_sp):
                    sp_chunks.append(dict(
                        sz=sp_sz, off=C_P + j * sp_sz, xv=None,
                        yv=ySp_v[0][:, :, j * sp_sz:(j + 1) * sp_sz],
                        hdt=BF16,
                        w1ap=sb_ap(w1s_sb[j]), w2ap=sb_ap(w2s_sb[j]),
                        b1t=b1s_sb[j], b2t=b2s_sb[j], prim=None,
                        xs=(j * sp_sz, (j + 1) * sp_sz)))
                if sp_last:
                    chunks = prim_chunks + sp_chunks
                else:
                    chunks = prim_chunks[:-1] + sp_chunks + prim_chunks[-1:]
                chunks[-1]["last"] = True
                nchk = len(chunks)

                def x_tile_for(i):
                    ch = chunks[i]
                    if ch["prim"] is None:
                        return xs_sb, ch["xs"]
                    if ch["prim"] == 0:
                        return x0, (0, ch["sz"])
                    if ch["prim"] == 1:
                        return x1, (0, ch["sz"])
                    xt = xpool.tile([128, KC, ch["sz"]], F32R, tag="x",
                                    name=f"x{i}")
                    for kc in range(0, KC, 2):
                        conv_x(xt, kc, ch["xv"][:, kc:kc + 2, :], ch["sz"])
                    return xt, (0, ch["sz"])

                def emit_h_mc(ch, mc, x_sb, xr, h_sb):
                    sz = ch["sz"]
                    ph = phpool.tile([128, NT], F32, tag="ph")
                    for kc in range(KC):
                        nc.tensor.matmul(
                            ph[:, :sz],
                            ch["w1ap"](kc, mc),
                            x_sb[:, kc, xr[0]:xr[1]],
                            start=(kc == 0), stop=(kc == KC - 1),
                        )
                    nc.vector.tensor_scalar(
                        h_sb[:, mc, :sz], ph[:, :sz],
                        ch["b1t"][:, mc:mc + 1], 0.0,
                        mybir.AluOpType.add, mybir.AluOpType.max,
                    )

                def emit_y_mc(ch, mc, h_sb, y_sb):
                    sz = ch["sz"]
                    py = pypool.tile([128, NT], F32, tag="py")
                    for kc in range(KC):
                        nc.tensor.matmul(
                            py[:, :sz],
                            ch["w2ap"](kc, mc),
                            h_sb[:, kc, :sz],
                            start=(kc == 0), stop=(kc == KC - 1),
                        )
                    nc.vector.tensor_scalar(
                        y_sb[:, mc, :sz], py[:, :sz],
                        ch["b2t"][:, mc:mc + 1], None,
                        mybir.AluOpType.add,
                    )
                    nc.vector.tensor_mul(
                        y_sb[:, mc, :sz], y_sb[:, mc, :sz],
                        wb_full[:, ch["off"]:ch["off"] + sz],
                    )
                    if ch.get("last"):
                        eng = (nc.sync, nc.scalar)[mc % 2]
                        if mc == MC - 1:
                            nc.sync.dma_start(ch["yv"][:, mc, 0:sz // 2],
                                              y_sb[:, mc, 0:sz // 2])
                            nc.scalar.dma_start(ch["yv"][:, mc, sz // 2:sz],
                                                y_sb[:, mc, sz // 2:sz])
                        else:
                            eng.dma_start(ch["yv"][:, mc, :],
                                          y_sb[:, mc, :sz])
                    else:
                        g = 4 if ch["prim"] is None else 2
                        if mc % g == g - 1:
                            nc.sync.dma_start(
                                ch["yv"][:, mc - g + 1:mc + 1, :],
                                y_sb[:, mc - g + 1:mc + 1, :sz])

                def emit_warmup():
                    # PE p-state warm-up: f32r matmuls on a memset tile —
                    # no DMA dependency, so the PE starts clocking up at
                    # ~0.5us and reaches 2.4GHz before the real work lands.
                    wmt = cpool.tile([128, NT], BF16, tag="wmt")
                    nc.vector.memset(wmt[:], 1.0)
                    pww = pypool.tile([128, NT], F32, tag="py", name="pww")
                    for i in range(n_warm):
                        nc.tensor.matmul(pww[0:1, :], wmt[:, 0:1], wmt[:],
                                         start=(i == 0), stop=(i == n_warm - 1))

                def emit_cold_h0():
                    ch = chunks[0]
                    h0 = hpool.tile([128, KC, ch["sz"]], BF16, tag="h",
                                    name="h0")
                    sz = ch["sz"]
                    for half in range(2):
                        phs = [phpool.tile([128, NT], F32, tag="ph",
                                           name=f"ph0_{half}_{m}")
                               for m in range(4)]
                        for kc in range(KC):
                            for m in range(4):
                                mc = half * 4 + m
                                nc.tensor.matmul(
                                    phs[m][:, :sz],
                                    w1stg[:, kc, bass.ts(mc, 128)],
                                    x0[:, kc, :sz],
                                    start=(kc == 0), stop=(kc == KC - 1),
                                )
                        for m in range(4):
                            mc = half * 4 + m
                            nc.vector.tensor_scalar(
                                h0[:, mc, :sz], phs[m][:, :sz],
                                b1_sb[:, mc:mc + 1], 0.0,
                                mybir.AluOpType.add, mybir.AluOpType.max,
                            )
                    return h0

                if not pipe:
                    emit_warmup()
                    for i, ch in enumerate(chunks):
                        if i == 0:
                            emit_x1_w2stg()
                            h_sb = emit_cold_h0()
                        else:
                            x_sb, xr = x_tile_for(i)
                            h_sb = hpool.tile([128, KC, ch["sz"]],
                                              ch["hdt"], tag="h")
                            for mc in range(MC):
                                emit_h_mc(ch, mc, x_sb, xr, h_sb)
                        if i == 1:
                            emit_converts_and_spill()
                        y_sb = ypool.tile([128, MC, ch["sz"]], BF16, tag="y")
                        for mc in range(MC):
                            emit_y_mc(ch, mc, h_sb, y_sb)
                else:
                    x_tiles = {}
                    h_tiles = {}
                    emit_warmup()
                    for s in range(nchk + 1):
                        if s == 0:
                            emit_x1_w2stg()
                            h_tiles[0] = emit_cold_h0()
                        if s < nchk:
                            ch = chunks[s]
                            if s + 1 < nchk:
                                x_tiles[s + 1] = x_tile_for(s + 1)
                            if s > 0:
                                h_tiles[s] = hpool.tile(
                                    [128, KC, ch["sz"]], ch["hdt"], tag="h",
                                    name=f"h{s}")
                        if s == 1:
                            emit_converts_and_spill()
                        y_sb = (ypool.tile([128, MC, chunks[s - 1]["sz"]],
                                           BF16, tag="y", name=f"y{s - 1}")
                                if s > 0 else None)
                        for mc in range(MC):
                            if 0 < s < nchk:
                                emit_h_mc(chunks[s], mc, *x_tiles[s],
                                          h_tiles[s])
                            if s > 0:
                                emit_y_mc(chunks[s - 1], mc,
                                          h_tiles[s - 1], y_sb)
                        x_tiles.pop(s - 1, None)
                        h_tiles.pop(s - 1, None)

    nc.compile()
    return nc


_NC_CACHE: dict = {}


def _get_kernel(n_sp: int, repeat: int = 1, **opts) -> bacc.Bacc:
    key = (n_sp, repeat, tuple(sorted(opts.items())))
    if key not in _NC_CACHE:
        _NC_CACHE[key] = build_moe_kernel(n_sp, repeat, **opts)
    return _NC_CACHE[key]


def dispatch(x, W_gate, b_gate):
    """Host-side gate + balanced top-2 dispatch plan."""
    xf = np.ascontiguousarray(np.asarray(x).reshape(-1, D), dtype=np.float32)
    scores = xf @ np.asarray(W_gate, np.float32) + np.asarray(b_gate, np.float32)
    top2 = np.argpartition(scores, N_EXPERTS - TOP_K, axis=1)[:, -TOP_K:]
    prim_ids, prim_wts, units, over = [], [], [], []
    for e in range(N_EXPERTS):
        tok = np.nonzero((top2 == e).any(axis=1))[0]
        prim_ids.append(tok[:C_P])
        prim_wts.append(scores[tok[:C_P], e])
        over.append(tok[C_P:])
    overs = [len(t) for t in over if len(t)]
    sp_sz = S_SP
    for cand in range(32, S_SP + 1, 32):
        if sum(-(-c // cand) for c in overs) <= N_CORES:
            sp_sz = cand
            break
    for e in range(N_EXPERTS):
        t_o = over[e]
        for i in range(0, len(t_o), sp_sz):
            t = t_o[i:i + sp_sz]
            units.append((e, t, scores[t, e]))
    n_sp = -(-len(units) // N_CORES) if units else 0
    core_units = [[] for _ in range(N_CORES)]
    for i, u in enumerate(units):
        core_units[i % N_CORES].append(u)
    empty = (0, np.zeros(0, np.int64), np.zeros(0, np.float32))
    for cu in core_units:
        while len(cu) < n_sp:
            cu.append(empty)
    return dict(xf=xf, prim_ids=prim_ids, prim_wts=prim_wts,
                core_units=core_units, n_sp=n_sp, sp_sz=sp_sz)


def make_in_maps(plan, parts):
    """Build per-core input dicts (chunk-major bf16 xT blocks)."""
    W1, b1, W2, b2 = parts
    W1b = W1.astype(NPBF)
    W2b = W2.astype(NPBF)
    xfb = plan["xf"].astype(NPBF)
    n_sp = plan["n_sp"]
    sp_sz = plan["sp_sz"]
    SP = n_sp * sp_sz
    C = C_P + SP
    in_maps = []
    for e in range(N_CORES):
        ids, wts = plan["prim_ids"][e], plan["prim_wts"][e]
        cnt = len(ids)
        xTe = np.zeros((D, C_P), np.float32)
        xTe[:, :cnt] = plan["xf"][ids].T
        wv = np.zeros((1, C), np.float32)
        wv[0, :cnt] = wts
        m = {
            "xT": xTe,
            "xB0": xTe[:, 0:SIZES[0]].astype(NPBF),
            "xB1": xTe[:, SIZES[0]:SIZES[0] + SIZES[1]].astype(NPBF),
            "w1": np.ascontiguousarray(W1b[e]), "b1": b1[e],
            "w2": np.ascontiguousarray(W2b[e]), "b2": b2[e],
        }
        if n_sp:
            xS = np.zeros((D, SP), NPBF)
            w1sl = np.zeros((n_sp, D, D), NPBF)
            w2sl = np.zeros((n_sp, D, D), NPBF)
            b1sl = np.zeros((n_sp, D), np.float32)
            b2sl = np.zeros((n_sp, D), np.float32)
            for j, (se, t, w) in enumerate(plan["core_units"][e]):
                xS[:, j * sp_sz:j * sp_sz + len(t)] = xfb[t].T
                wv[0, C_P + j * sp_sz:C_P + j * sp_sz + len(t)] = w
                w1sl[j] = W1b[se]
                w2sl[j] = W2b[se]
                b1sl[j] = b1[se]
                b2sl[j] = b2[se]
            m.update(xSp=np.ascontiguousarray(xS), w1s=w1sl, w2s=w2sl,
                     b1s=b1sl, b2s=b2sl)
        m["wbT"] = np.ascontiguousarray(
            np.broadcast_to(wv, (128, C))).astype(NPBF)
        in_maps.append(m)
    return in_maps


def kernel(x, W_gate, b_gate, W1, b1, W2, b2):
    plan = dispatch(x, W_gate, b_gate)
    nc = _get_kernel(plan["n_sp"], sp_sz=plan["sp_sz"])

    parts = (np.asarray(W1, np.float32), np.asarray(b1, np.float32),
             np.asarray(W2, np.float32), np.asarray(b2, np.float32))
    in_maps = make_in_maps(plan, parts)

    res = run_bass_kernel_spmd(nc, in_maps, core_ids=list(range(N_CORES)))

    out = np.zeros((N_TOKENS, D), np.float32)
    for e in range(N_CORES):
        r = res.results[e]
        ids = plan["prim_ids"][e]
        yTe = r["yT"].astype(np.float32)
        out[ids] += yTe.T[:len(ids)]
        if plan["n_sp"]:
            yS = r["ySp"][0].astype(np.float32)
            sp_sz = plan["sp_sz"]
            for j, (se, t, w) in enumerate(plan["core_units"][e]):
                if len(t):
                    out[t] += yS[:, j * sp_sz:j * sp_sz + len(t)].T
    return out.reshape(B, T, D)
